# revision 23
# baseline (speedup 1.0000x reference)
"""Trainium2 Bass kernel for nn_KnowledgeFusion.

Math (b=8, H=W=32, d=o=256, n_obj=15):
  embs_aug = concat([embs, mean(embs)])                  [b,16,256]
  mask     = rasterized boxes (rounded to PATCH_SIZE=2)  [b,16,1024] in {0,1}
  proj     = patches @ Wp                                [b,1024,256]
  inj      = embs_aug @ We                               [b,16,256]
  s[hw]    = sum_n mask[n,hw]   (>=1: image box row)
  out      = proj + (mask^T @ inj) / s[:,None]           [b,1024,256]

The mean-emb row folds away: with inj_k = embs_k @ We (k<15),
  sum_{n<16} maskN[n] inj_n = sum_{k<15} (mask_k + 1/15) * recB * inj_k
since the image-box row has mask=1 everywhere, so the whole kernel is
  outT[o,hw] = Wp^T @ patchesT + inj^T @ ((mask + 1/15) * recB)

recB = 1/s computed exactly without any reciprocal: s is an integer in
1..16, so partition p of the replicated-s PSUM tile tests s == p+1
(one is_equal with a per-partition constant), and a [16,16] matmul
against weights 1/(p+1) collapses the one-hot back to 1/s. All ACT-
engine ops are plain Copy, so exactly one activation-table load fires,
off the critical path.

Everything is bf16 (inputs cast on host, output upcast on host) to
halve HBM traffic; rel-err lands ~4e-3 against the 2e-2 gate.

Per-core schedule (one batch element per core):
  sync queue:   loc (256B, heads the longest dep chain), then wb
  scalar queue: pT half 0, pT half 1
  PE order interleaves proj matmuls (gated on pT) with the mask-chain
  matmuls (gated on loc) so each of the four PSUM banks closes as early
  as possible; each bank evacuates to bf16 (DVE/ACT alternating) and
  leaves through its own output DMA immediately.
"""

import sys

sys.path.insert(0, "/opt/trn_rl_repo")

import numpy as np

import concourse.bass as bass
import concourse.bacc as bacc
import concourse.mybir as mybir
from concourse import tile
from concourse import bass_utils
from concourse.alu_op_type import AluOpType

B, H, W, D = 8, 32, 32, 256
NOBJ, N = 15, 16
HW = H * W
O = 256
FP = mybir.dt.float32
BF = mybir.dt.bfloat16
I32 = mybir.dt.int32
AF = mybir.ActivationFunctionType

# weights blob columns (bf16): Wp0 Wp1 We0 We1 eT0 eT1 pad
WB = 4 * O + 2 * NOBJ + 2  # 1056


def _ap(ap, free_dims):
    """AP with explicit free-dim [step, count] pairs (step 0 = broadcast)."""
    return bass.AP(ap.tensor, ap.offset, ap.ap[:1] + free_dims)


def build_nc(debug: bool = False):
    nc = bacc.Bacc("TRN2", target_bir_lowering=False, debug=debug, num_devices=B)

    loc = nc.dram_tensor("loc", [N, 4], I32, kind="ExternalInput")
    wb = nc.dram_tensor("wb", [128, WB], BF, kind="ExternalInput")
    pT = nc.dram_tensor("pT", [128, 2 * HW], BF, kind="ExternalInput")
    outT = nc.dram_tensor("outT", [128, 2 * HW], BF, kind="ExternalOutput")

    with tile.TileContext(nc) as tc:
        with (
            nc.allow_low_precision(reason="bf16 matmuls, fp32 PSUM accumulation"),
            tc.tile_pool(name="big", bufs=1) as big,
            tc.tile_pool(name="small", bufs=1) as small,
            tc.tile_pool(name="outp", bufs=1) as outp,
            tc.tile_pool(name="psT", bufs=1, space=bass.MemorySpace.PSUM) as psT,
            tc.tile_pool(name="psS", bufs=1, space=bass.MemorySpace.PSUM) as psS,
            # psumI and psumR1 share one bank slot (disjoint lifetimes)
            tc.tile_pool(name="psI", bufs=1, space=bass.MemorySpace.PSUM) as psI,
        ):
            # ---- input DMAs, finest useful granularity and balanced
            # across the two HWDGE queues, ordered by first use:
            #   sync:   loc (heads the mask chain), pT k=1, pT k=0/h0
            #   scalar: We+eT (unblocks injpre), Wp, pT k=0/h1
            loc_sb = small.tile([N, 4], I32)
            nc.sync.dma_start(loc_sb[:], loc[:])
            wb_sb = big.tile([128, WB], BF)
            nc.scalar.dma_start(wb_sb[:, 2 * O : WB], wb[:, 2 * O : WB])  # We+eT
            pT_sb = big.tile([128, 2 * HW], BF)
            nc.sync.dma_start(pT_sb[:, HW : 2 * HW], pT[:, HW : 2 * HW])  # k=1
            nc.scalar.dma_start(wb_sb[:, 0 : 2 * O], wb[:, 0 : 2 * O])  # Wp
            nc.sync.dma_start(pT_sb[:, 0:512], pT[:, 0:512])  # k=0 h=0
            nc.scalar.dma_start(pT_sb[:, 512:HW], pT[:, 512:HW])  # k=0 h=1

            Wp_sb = [wb_sb[:, O * k : O * (k + 1)] for k in range(2)]
            We_sb = [wb_sb[:, 2 * O + O * k : 2 * O + O * (k + 1)] for k in range(2)]
            eT_sb = [
                wb_sb[:, 4 * O + NOBJ * k : 4 * O + NOBJ * (k + 1)] for k in range(2)
            ]

            # ---- constants (all off the critical path)
            ones16 = small.tile([N, N], BF, name="ones16")
            nc.gpsimd.memset(ones16[:], 1.0)
            grid_i = small.tile([N, 32], I32, name="grid")
            nc.gpsimd.iota(grid_i[:], pattern=[[1, 32]], base=0, channel_multiplier=0)
            grid_f = small.tile([N, 32], FP, name="gridf")
            nc.vector.tensor_copy(grid_f[:], grid_i[:])
            grid2_f = small.tile([N, 32], FP, name="grid2f")
            nc.vector.tensor_scalar(
                grid2_f[:], grid_f[:], 2.0, None, op0=AluOpType.subtract
            )
            kidx = small.tile([N, 1], I32, name="kidx")
            nc.gpsimd.iota(kidx[:], pattern=[[1, 1]], base=1, channel_multiplier=1)
            kvec = small.tile([N, 1], FP, name="kvec")
            nc.vector.tensor_copy(kvec[:], kidx[:])
            wn = small.tile([N, 1], FP, name="wn")
            nc.vector.reciprocal(wn[:], kvec[:])
            w16 = small.tile([N, N], BF, name="w16")
            nc.vector.tensor_copy(w16[:], _ap(wn[:], [[0, N]]))

            # ---- boxes: round starts down; ends handled via shifted grid
            boxes_i = small.tile([N, 4], I32, name="boxes_i")
            nc.vector.tensor_scalar(
                boxes_i[:], loc_sb[:], -2, None, op0=AluOpType.bitwise_and
            )
            boxes = small.tile([N, 4], FP, name="boxes")
            nc.vector.tensor_copy(boxes[:], boxes_i[:])

            # ---- row/col interval masks [16, 32] (bf16 0/1)
            rowm = small.tile([N, 32], BF, name="rowm")
            colm = small.tile([N, 32], BF, name="colm")
            tmp_y = small.tile([N, 32], FP, name="tmp_y")
            tmp_x = small.tile([N, 32], FP, name="tmp_x")
            # grid-2 < (end&-2)  ==  grid < (end&-2)+2
            nc.vector.tensor_scalar(
                tmp_y[:], grid2_f[:], boxes[:, 2:3], None, op0=AluOpType.is_lt
            )
            nc.vector.scalar_tensor_tensor(
                rowm[:], grid_f[:], boxes[:, 0:1], tmp_y[:],
                op0=AluOpType.is_ge, op1=AluOpType.mult,
            )
            nc.vector.tensor_scalar(
                tmp_x[:], grid2_f[:], boxes[:, 3:4], None, op0=AluOpType.is_lt
            )
            nc.vector.scalar_tensor_tensor(
                colm[:], grid_f[:], boxes[:, 1:2], tmp_x[:],
                op0=AluOpType.is_ge, op1=AluOpType.mult,
            )

            # ---- per-half mask chain tiles
            mask = small.tile([N, HW], BF, name="mask")
            ind = small.tile([N, HW], BF, name="ind")
            maskN = small.tile([N, HW], BF, name="maskN")
            psumS = [psS.tile([N, 512], FP, name=f"psS{h}") for h in range(2)]
            psumI = psI.tile([NOBJ, 512], FP, tag="psi", name="psI")
            psumR = [
                psS.tile([N, 512], FP, name="psR0"),
                psI.tile([N, 512], FP, tag="psi", name="psR1"),
            ]
            psum = [[psT.tile([128, 512], FP, name=f"ps{h}{oc}") for oc in range(2)]
                    for h in range(2)]

            def mask_half(h):
                # mask[:, h*512:(h+1)*512] = rowm[:, h*16:+16] x colm  (outer)
                nc.vector.tensor_tensor(
                    _ap(mask[:, 512 * h : 512 * (h + 1)], [[W, 16], [1, W]]),
                    _ap(rowm[:, 16 * h : 16 * (h + 1)], [[1, 16], [0, W]]),
                    _ap(colm[:], [[0, 16], [1, W]]),
                    op=AluOpType.mult,
                )

            def s_half(h):  # s replicated over the 16 partitions
                return nc.tensor.matmul(
                    psumS[h][:], ones16[:], mask[:, 512 * h : 512 * (h + 1)],
                    start=True, stop=True,
                )

            def iseq_half(h):  # partition p: ind = (s == p+1)
                nc.vector.tensor_scalar(
                    ind[:, 512 * h : 512 * (h + 1)], psumS[h][:], kvec[:, 0:1],
                    None, op0=AluOpType.is_equal,
                )

            def ind_mm_half(h):  # recB = w16^T @ ind = 1/s (replicated)
                return nc.tensor.matmul(
                    psumR[h][:], w16[:], ind[:, 512 * h : 512 * (h + 1)],
                    start=True, stop=True,
                )

            def maskN_half(h):
                # (mask + 1/15) * recB  -- the +1/15 carries the mean-emb row
                nc.vector.scalar_tensor_tensor(
                    maskN[:, 512 * h : 512 * (h + 1)],
                    mask[:, 512 * h : 512 * (h + 1)],
                    1.0 / NOBJ,
                    psumR[h][:],
                    op0=AluOpType.add, op1=AluOpType.mult,
                )

            def proj_mm(h, oc, k, start):
                return nc.tensor.matmul(
                    psum[h][oc][:],
                    Wp_sb[k][:, 128 * oc : 128 * (oc + 1)],
                    pT_sb[:, HW * k + 512 * h : HW * k + 512 * (h + 1)],
                    start=start, stop=False,
                )

            def inj_mm(h, oc):
                return nc.tensor.matmul(
                    psum[h][oc][:],
                    inj_sb[:, 128 * oc : 128 * (oc + 1)],
                    maskN[0:NOBJ, 512 * h : 512 * (h + 1)],
                    start=False, stop=True,
                )

            # ---- emission order doubles as per-engine FIFO order and
            # MUST be topological: Tile tracks deps by trace order, so
            # every consumer is emitted after its producer.
            mask_half(0)
            mask_half(1)

            # inj = embs @ We (gated on We+eT only)
            inj_pre0 = nc.tensor.matmul(
                psumI[:, 0:O], eT_sb[0][:], We_sb[0][:], start=True, stop=False
            )
            inj_pre1 = nc.tensor.matmul(
                psumI[:, 0:O], eT_sb[1][:], We_sb[1][:], start=False, stop=True
            )
            inj_sb = small.tile([NOBJ, O], BF, name="inj")
            nc.scalar.activation(inj_sb[:], psumI[:, 0:O], AF.Copy)

            # The PE order is forced with explicit dependency edges --
            # the Tile scheduler's cost model otherwise spreads the tiny
            # chain matmuls (s, ind) between the proj matmuls, which
            # delays maskN and the bank-closing inj matmuls by ~2us.
            # Order: chosen by expected operand-landing times; proj k=1
            # (first pT half to land) opens each bank, inj closes it.
            pe = [inj_pre0, inj_pre1]
            pe.append(proj_mm(0, 0, 1, True))
            pe.append(proj_mm(0, 1, 1, True))
            pe.append(s_half(0))
            pe.append(s_half(1))
            iseq_half(0)
            iseq_half(1)
            pe.append(proj_mm(1, 0, 1, True))
            pe.append(proj_mm(1, 1, 1, True))
            pe.append(ind_mm_half(0))
            pe.append(ind_mm_half(1))
            maskN_half(0)
            maskN_half(1)
            pe.append(proj_mm(0, 0, 0, False))
            pe.append(inj_mm(0, 0))
            pe.append(proj_mm(0, 1, 0, False))
            pe.append(inj_mm(0, 1))
            pe.append(proj_mm(1, 0, 0, False))
            pe.append(inj_mm(1, 0))
            pe.append(proj_mm(1, 1, 0, False))
            pe.append(inj_mm(1, 1))
            for a, b in zip(pe, pe[1:]):
                tile.add_dep_helper(b.ins, a.ins, sync=False, reason="PE order")

            # ---- evacuate + store per bank: out col layout oc*1024 + h*512
            # (DVE is busy with maskN until late -- ACT takes 3 of 4)
            o_sb = outp.tile([128, 2 * HW], BF, name="osb")
            for idx, (h, oc) in enumerate([(0, 0), (0, 1), (1, 0), (1, 1)]):
                c0 = 1024 * oc + 512 * h
                dst = o_sb[:, c0 : c0 + 512]
                if idx == 2:
                    nc.vector.tensor_copy(dst, psum[h][oc][:])
                    nc.sync.dma_start(outT[:, c0 : c0 + 512], dst)
                else:
                    nc.scalar.activation(dst, psum[h][oc][:], AF.Copy)
                    eng = nc.sync if idx == 0 else nc.scalar
                    eng.dma_start(outT[:, c0 : c0 + 512], dst)

    nc.compile()
    return nc


def make_in_maps(inputs):
    import ml_dtypes

    bf16 = ml_dtypes.bfloat16
    patches = np.asarray(inputs["patches"], dtype=np.float32)
    embs = np.asarray(inputs["embs"], dtype=np.float32)
    locations = np.asarray(inputs["locations"], dtype=np.int32)
    Wp = np.asarray(inputs["Wp"], dtype=np.float32)
    We = np.asarray(inputs["We"], dtype=np.float32)
    img_box = np.array([[0, 0, H, W]], dtype=np.int32)
    wb_common = np.zeros((128, WB), dtype=np.float32)
    wb_common[:, 0:O] = Wp[0:128]
    wb_common[:, O : 2 * O] = Wp[128:256]
    wb_common[:, 2 * O : 3 * O] = We[0:128]
    wb_common[:, 3 * O : 4 * O] = We[128:256]
    in_maps = []
    for b in range(B):
        eTb = embs[b].T  # [256, 15]
        wbb = wb_common.copy()
        wbb[:, 4 * O : 4 * O + NOBJ] = eTb[0:128]
        wbb[:, 4 * O + NOBJ : 4 * O + 2 * NOBJ] = eTb[128:256]
        pTb = patches[b].reshape(HW, D).T  # [256, 1024]
        pT2 = np.concatenate([pTb[0:128], pTb[128:256]], axis=1)  # [128, 2048]
        in_maps.append(
            {
                "loc": np.ascontiguousarray(np.concatenate([locations[b], img_box], 0)),
                "wb": np.ascontiguousarray(wbb.astype(bf16)),
                "pT": np.ascontiguousarray(pT2.astype(bf16)),
            }
        )
    return in_maps


_NC = None


def _get_nc():
    global _NC
    if _NC is None:
        _NC = build_nc(debug=False)
    return _NC


def run(inputs, trace: bool = False, **kwargs):
    nc = _get_nc()
    res = bass_utils.run_bass_kernel_spmd(
        nc, make_in_maps(inputs), core_ids=list(range(B)), trace=trace, **kwargs
    )
    outs = []
    for b in range(B):
        arr = np.asarray(res.results[b]["outT"]).astype(np.float32)  # [128, 2048]
        outs.append(np.concatenate([arr[:, 0:HW].T, arr[:, HW : 2 * HW].T], axis=1))
    full = np.stack(outs, axis=0)
    return np.ascontiguousarray(full).astype(np.float32), res


def kernel(**inputs) -> np.ndarray:
    full, _ = run(inputs, trace=False)
    return full


# revision 26
# speedup vs baseline: 1.0672x; 1.0672x over previous
"""Trainium2 Bass kernel for nn_KnowledgeFusion.

Math (b=8, H=W=32, d=o=256, n_obj=15):
  embs_aug = concat([embs, mean(embs)])                  [b,16,256]
  mask     = rasterized boxes (rounded to PATCH_SIZE=2)  [b,16,1024] in {0,1}
  proj     = patches @ Wp                                [b,1024,256]
  inj      = embs_aug @ We                               [b,16,256]
  s[hw]    = sum_n mask[n,hw]   (>=1: image box row)
  out      = proj + (mask^T @ inj) / s[:,None]           [b,1024,256]

The mean-emb row folds away: with inj_k = embs_k @ We (k<15),
  sum_{n<16} maskN[n] inj_n = sum_{k<15} (mask_k + 1/15) * recB * inj_k
since the image-box row has mask=1 everywhere, so the whole kernel is
  outT[o,hw] = Wp^T @ patchesT + inj^T @ ((mask + 1/15) * recB)

recB = 1/s computed exactly without any reciprocal: s is an integer in
1..16, so partition p of the replicated-s PSUM tile tests s == p+1
(one is_equal with a per-partition constant), and a [16,16] matmul
against weights 1/(p+1) collapses the one-hot back to 1/s. All ACT-
engine ops are plain Copy, so exactly one activation-table load fires,
off the critical path.

Everything is bf16 (inputs cast on host, output upcast on host) to
halve HBM traffic; rel-err lands ~4e-3 against the 2e-2 gate.

Per-core schedule (one batch element per core):
  sync queue:   loc (256B, heads the longest dep chain), then wb
  scalar queue: pT half 0, pT half 1
  PE order interleaves proj matmuls (gated on pT) with the mask-chain
  matmuls (gated on loc) so each of the four PSUM banks closes as early
  as possible; each bank evacuates to bf16 (DVE/ACT alternating) and
  leaves through its own output DMA immediately.
"""

import sys

sys.path.insert(0, "/opt/trn_rl_repo")

import numpy as np

import concourse.bass as bass
import concourse.bacc as bacc
import concourse.mybir as mybir
from concourse import tile
from concourse import bass_utils
from concourse.alu_op_type import AluOpType

B, H, W, D = 8, 32, 32, 256
NOBJ, N = 15, 16
HW = H * W
O = 256
FP = mybir.dt.float32
BF = mybir.dt.bfloat16
I32 = mybir.dt.int32
AF = mybir.ActivationFunctionType

# weights blob columns (bf16): Wp0 Wp1 We0 We1 eT0 eT1 pad
WB = 4 * O + 2 * NOBJ + 2  # 1056


def _ap(ap, free_dims):
    """AP with explicit free-dim [step, count] pairs (step 0 = broadcast)."""
    return bass.AP(ap.tensor, ap.offset, ap.ap[:1] + free_dims)


def build_nc(debug: bool = False):
    nc = bacc.Bacc("TRN2", target_bir_lowering=False, debug=debug, num_devices=B)

    loc = nc.dram_tensor("loc", [N, 4], I32, kind="ExternalInput")
    wb = nc.dram_tensor("wb", [128, WB], BF, kind="ExternalInput")
    pT = nc.dram_tensor("pT", [128, 2 * HW], BF, kind="ExternalInput")
    outT = nc.dram_tensor("outT", [128, 2 * HW], BF, kind="ExternalOutput")

    with tile.TileContext(nc) as tc:
        with (
            nc.allow_low_precision(reason="bf16 matmuls, fp32 PSUM accumulation"),
            tc.tile_pool(name="big", bufs=1) as big,
            tc.tile_pool(name="small", bufs=1) as small,
            tc.tile_pool(name="outp", bufs=1) as outp,
            tc.tile_pool(name="psT", bufs=1, space=bass.MemorySpace.PSUM) as psT,
            tc.tile_pool(name="psS", bufs=1, space=bass.MemorySpace.PSUM) as psS,
            # psumI and psumR1 share one bank slot (disjoint lifetimes)
            tc.tile_pool(name="psI", bufs=1, space=bass.MemorySpace.PSUM) as psI,
        ):
            # ---- input DMAs. The input direction is HBM-read-latency
            # bound (~210 GB/s aggregate over both HWDGE queues no
            # matter the descriptor shape), so just order by first use:
            #   sync:   loc (heads the mask chain), pT k=0
            #   scalar: wb (unblocks injpre + Wp), pT k=1
            loc_sb = small.tile([N, 4], I32)
            nc.sync.dma_start(loc_sb[:], loc[:])
            wb_sb = big.tile([128, WB], BF)
            nc.scalar.dma_start(wb_sb[:], wb[:])
            pT_sb = big.tile([128, 2 * HW], BF)
            nc.sync.dma_start(pT_sb[:, 0:HW], pT[:, 0:HW])  # k=0
            nc.scalar.dma_start(pT_sb[:, HW : 2 * HW], pT[:, HW : 2 * HW])  # k=1

            Wp_sb = [wb_sb[:, O * k : O * (k + 1)] for k in range(2)]
            We_sb = [wb_sb[:, 2 * O + O * k : 2 * O + O * (k + 1)] for k in range(2)]
            eT_sb = [
                wb_sb[:, 4 * O + NOBJ * k : 4 * O + NOBJ * (k + 1)] for k in range(2)
            ]

            # ---- constants (all off the critical path)
            ones16 = small.tile([N, N], BF, name="ones16")
            nc.gpsimd.memset(ones16[:], 1.0)
            grid_i = small.tile([N, 32], I32, name="grid")
            nc.gpsimd.iota(grid_i[:], pattern=[[1, 32]], base=0, channel_multiplier=0)
            grid_f = small.tile([N, 32], FP, name="gridf")
            nc.vector.tensor_copy(grid_f[:], grid_i[:])
            grid2_f = small.tile([N, 32], FP, name="grid2f")
            nc.vector.tensor_scalar(
                grid2_f[:], grid_f[:], 2.0, None, op0=AluOpType.subtract
            )
            kidx = small.tile([N, 1], I32, name="kidx")
            nc.gpsimd.iota(kidx[:], pattern=[[1, 1]], base=1, channel_multiplier=1)
            kvec = small.tile([N, 1], FP, name="kvec")
            nc.vector.tensor_copy(kvec[:], kidx[:])
            wn = small.tile([N, 1], FP, name="wn")
            nc.vector.reciprocal(wn[:], kvec[:])
            w16 = small.tile([N, N], BF, name="w16")
            nc.vector.tensor_copy(w16[:], _ap(wn[:], [[0, N]]))

            # ---- boxes: round starts down; ends handled via shifted grid
            boxes_i = small.tile([N, 4], I32, name="boxes_i")
            nc.vector.tensor_scalar(
                boxes_i[:], loc_sb[:], -2, None, op0=AluOpType.bitwise_and
            )
            boxes = small.tile([N, 4], FP, name="boxes")
            nc.vector.tensor_copy(boxes[:], boxes_i[:])

            # ---- row/col interval masks [16, 32] (bf16 0/1)
            rowm = small.tile([N, 32], BF, name="rowm")
            colm = small.tile([N, 32], BF, name="colm")
            tmp_y = small.tile([N, 32], FP, name="tmp_y")
            tmp_x = small.tile([N, 32], FP, name="tmp_x")
            # grid-2 < (end&-2)  ==  grid < (end&-2)+2
            nc.vector.tensor_scalar(
                tmp_y[:], grid2_f[:], boxes[:, 2:3], None, op0=AluOpType.is_lt
            )
            nc.vector.scalar_tensor_tensor(
                rowm[:], grid_f[:], boxes[:, 0:1], tmp_y[:],
                op0=AluOpType.is_ge, op1=AluOpType.mult,
            )
            nc.vector.tensor_scalar(
                tmp_x[:], grid2_f[:], boxes[:, 3:4], None, op0=AluOpType.is_lt
            )
            nc.vector.scalar_tensor_tensor(
                colm[:], grid_f[:], boxes[:, 1:2], tmp_x[:],
                op0=AluOpType.is_ge, op1=AluOpType.mult,
            )

            # ---- per-half mask chain tiles
            mask = small.tile([N, HW], BF, name="mask")
            ind = small.tile([N, HW], BF, name="ind")
            maskN = small.tile([N, HW], BF, name="maskN")
            psumS = [psS.tile([N, 512], FP, name=f"psS{h}") for h in range(2)]
            psumI = psI.tile([NOBJ, 512], FP, tag="psi", name="psI")
            psumR = [
                psS.tile([N, 512], FP, name="psR0"),
                psI.tile([N, 512], FP, tag="psi", name="psR1"),
            ]
            psum = [[psT.tile([128, 512], FP, name=f"ps{h}{oc}") for oc in range(2)]
                    for h in range(2)]

            def mask_half(h):
                # mask[:, h*512:(h+1)*512] = rowm[:, h*16:+16] x colm  (outer)
                nc.vector.tensor_tensor(
                    _ap(mask[:, 512 * h : 512 * (h + 1)], [[W, 16], [1, W]]),
                    _ap(rowm[:, 16 * h : 16 * (h + 1)], [[1, 16], [0, W]]),
                    _ap(colm[:], [[0, 16], [1, W]]),
                    op=AluOpType.mult,
                )

            def s_half(h):  # s replicated over the 16 partitions
                return nc.tensor.matmul(
                    psumS[h][:], ones16[:], mask[:, 512 * h : 512 * (h + 1)],
                    start=True, stop=True,
                )

            def iseq_half(h):  # partition p: ind = (s == p+1)
                nc.vector.tensor_scalar(
                    ind[:, 512 * h : 512 * (h + 1)], psumS[h][:], kvec[:, 0:1],
                    None, op0=AluOpType.is_equal,
                )

            def ind_mm_half(h):  # recB = w16^T @ ind = 1/s (replicated)
                return nc.tensor.matmul(
                    psumR[h][:], w16[:], ind[:, 512 * h : 512 * (h + 1)],
                    start=True, stop=True,
                )

            def maskN_half(h):
                # (mask + 1/15) * recB  -- the +1/15 carries the mean-emb row
                nc.vector.scalar_tensor_tensor(
                    maskN[:, 512 * h : 512 * (h + 1)],
                    mask[:, 512 * h : 512 * (h + 1)],
                    1.0 / NOBJ,
                    psumR[h][:],
                    op0=AluOpType.add, op1=AluOpType.mult,
                )

            def proj_mm(h, oc, k, start):
                return nc.tensor.matmul(
                    psum[h][oc][:],
                    Wp_sb[k][:, 128 * oc : 128 * (oc + 1)],
                    pT_sb[:, HW * k + 512 * h : HW * k + 512 * (h + 1)],
                    start=start, stop=False,
                )

            def inj_mm(h, oc):
                return nc.tensor.matmul(
                    psum[h][oc][:],
                    inj_sb[:, 128 * oc : 128 * (oc + 1)],
                    maskN[0:NOBJ, 512 * h : 512 * (h + 1)],
                    start=False, stop=True,
                )

            # ---- emission order doubles as per-engine FIFO order and
            # MUST be topological: Tile tracks deps by trace order, so
            # every consumer is emitted after its producer.
            mask_half(0)
            mask_half(1)

            # inj = embs @ We (gated on We+eT only)
            inj_pre0 = nc.tensor.matmul(
                psumI[:, 0:O], eT_sb[0][:], We_sb[0][:], start=True, stop=False
            )
            inj_pre1 = nc.tensor.matmul(
                psumI[:, 0:O], eT_sb[1][:], We_sb[1][:], start=False, stop=True
            )
            inj_sb = small.tile([NOBJ, O], BF, name="inj")
            nc.scalar.activation(inj_sb[:], psumI[:, 0:O], AF.Copy)

            # The PE order is forced with explicit dependency edges --
            # the Tile scheduler's cost model otherwise spreads the tiny
            # chain matmuls (s, ind) between the proj matmuls, which
            # delays maskN and the bank-closing inj matmuls by ~2us.
            # Order: chosen by expected operand-landing times; proj k=1
            # (first pT half to land) opens each bank, inj closes it.
            pe = [inj_pre0, inj_pre1]
            pe.append(s_half(0))
            pe.append(s_half(1))
            iseq_half(0)
            iseq_half(1)
            pe.append(proj_mm(0, 0, 0, True))
            pe.append(proj_mm(0, 1, 0, True))
            pe.append(proj_mm(1, 0, 0, True))
            pe.append(proj_mm(1, 1, 0, True))
            pe.append(ind_mm_half(0))
            pe.append(ind_mm_half(1))
            maskN_half(0)
            maskN_half(1)
            pe.append(proj_mm(0, 0, 1, False))
            pe.append(proj_mm(0, 1, 1, False))
            pe.append(inj_mm(0, 0))
            pe.append(inj_mm(0, 1))
            pe.append(proj_mm(1, 0, 1, False))
            pe.append(proj_mm(1, 1, 1, False))
            pe.append(inj_mm(1, 0))
            pe.append(inj_mm(1, 1))
            for a, b in zip(pe, pe[1:]):
                tile.add_dep_helper(b.ins, a.ins, sync=False, reason="PE order")

            # ---- evacuate + store per bank in closure order, evacs
            # alternating ACT/DVE, each bank leaving through its own DMA
            o_sb = outp.tile([128, 2 * HW], BF, name="osb")
            for idx, (h, oc) in enumerate([(0, 0), (0, 1), (1, 0), (1, 1)]):
                c0 = 1024 * oc + 512 * h
                dst = o_sb[:, c0 : c0 + 512]
                if idx % 2 == 0:
                    nc.scalar.activation(dst, psum[h][oc][:], AF.Copy)
                else:
                    nc.vector.tensor_copy(dst, psum[h][oc][:])
                eng = nc.sync if idx % 2 == 0 else nc.scalar
                eng.dma_start(outT[:, c0 : c0 + 512], dst)

    nc.compile()
    return nc


def make_in_maps(inputs):
    import ml_dtypes

    bf16 = ml_dtypes.bfloat16
    patches = np.asarray(inputs["patches"], dtype=np.float32)
    embs = np.asarray(inputs["embs"], dtype=np.float32)
    locations = np.asarray(inputs["locations"], dtype=np.int32)
    Wp = np.asarray(inputs["Wp"], dtype=np.float32)
    We = np.asarray(inputs["We"], dtype=np.float32)
    img_box = np.array([[0, 0, H, W]], dtype=np.int32)
    wb_common = np.zeros((128, WB), dtype=np.float32)
    wb_common[:, 0:O] = Wp[0:128]
    wb_common[:, O : 2 * O] = Wp[128:256]
    wb_common[:, 2 * O : 3 * O] = We[0:128]
    wb_common[:, 3 * O : 4 * O] = We[128:256]
    in_maps = []
    for b in range(B):
        eTb = embs[b].T  # [256, 15]
        wbb = wb_common.copy()
        wbb[:, 4 * O : 4 * O + NOBJ] = eTb[0:128]
        wbb[:, 4 * O + NOBJ : 4 * O + 2 * NOBJ] = eTb[128:256]
        pTb = patches[b].reshape(HW, D).T  # [256, 1024]
        pT2 = np.concatenate([pTb[0:128], pTb[128:256]], axis=1)  # [128, 2048]
        in_maps.append(
            {
                "loc": np.ascontiguousarray(np.concatenate([locations[b], img_box], 0)),
                "wb": np.ascontiguousarray(wbb.astype(bf16)),
                "pT": np.ascontiguousarray(pT2.astype(bf16)),
            }
        )
    return in_maps


_NC = None


def _get_nc():
    global _NC
    if _NC is None:
        _NC = build_nc(debug=False)
    return _NC


def run(inputs, trace: bool = False, **kwargs):
    nc = _get_nc()
    res = bass_utils.run_bass_kernel_spmd(
        nc, make_in_maps(inputs), core_ids=list(range(B)), trace=trace, **kwargs
    )
    outs = []
    for b in range(B):
        arr = np.asarray(res.results[b]["outT"]).astype(np.float32)  # [128, 2048]
        outs.append(np.concatenate([arr[:, 0:HW].T, arr[:, HW : 2 * HW].T], axis=1))
    full = np.stack(outs, axis=0)
    return np.ascontiguousarray(full).astype(np.float32), res


def kernel(**inputs) -> np.ndarray:
    full, _ = run(inputs, trace=False)
    return full
